# revision 9
# baseline (speedup 1.0000x reference)
"""Trainium2 Bass kernel for a 12-head attention block (B=2, N=2048, C=768).

Sharding: the 24 (batch, head) pairs are split across 8 NeuronCores —
4 cores per batch element, 3 heads per core (data + head/tensor parallel).
Each core computes qkv projections for its heads, the full attention for
its heads (the N x N score matrix is private to a core), and a *partial*
output projection over its heads' channels.  The host sums the 4 partial
projections per batch element (the tensor-parallel all-reduce) and adds
the bias.

Device algorithm (activations/weights bf16, fp32 PSUM accumulation):

  xT [768, 2048] (x transposed on host)
  B:  qk^T  = W_qk^T.T @ xT  -> per-head tile [q^T(64 rows); k^T(64)] x 2048
      (attention scale 1/8 and b_q, b_k folded into W/bias on host)
  B2: v     = xT.T @ W_v^T   -> [2048, 3*65] with a column of ones per head
  C:  S^T[key, q] = k^T.T @ q^T        (per 128-key tile, 512-q chunk)
      P^T = exp(S^T)                   (ScalarE, no max subtraction:
                                        logits are in [-3, 3] by construction)
      ctx_u^T[d|den, q] += [v | 1].T @ P^T   (fused denominator row)
  D:  ctx^T = ctx_u^T[0:64] * (1/den)  (reciprocal on DVE, den row shifted to
      partition 0 by a tiny SBUF DMA, broadcast across partitions on the
      otherwise-idle GPSIMD engine, then one tensor_mul)
  E:  y[n, :] += ctx^T.T @ W_p^T      (partial projection, summed on host)

Scheduling notes: PE tile-config switches (64x128 <-> 128x128 <-> 128x65)
cost ~107ns of array drain each, so matmuls are emitted in same-shape
groups covering two key-tile pairs at a time: [S x4][proj/E][qk or v][PV x4],
with the PV consumers lagging one group behind their exp.  The PE warm-up
spin runs on a memset tile so it needs no DMA and starts immediately; the
exp table set is preloaded the same way.  x arrives in four column-waves so
the first qk projection (and the first S-pairs) start ~3us in.
"""

import numpy as np
import ml_dtypes

import concourse.bacc as bacc
import concourse.tile as tile
import concourse.mybir as mybir
from concourse.bass_utils import run_bass_kernel_spmd

# Problem shape (hardcoded; harness contract)
B, N, C = 2, 2048, 768
H, HD = 12, 64
NCORES = 8
CORES_PER_B = NCORES // B      # 4
HPC = H // CORES_PER_B         # 3 heads per core
P = 128
NT = N // P                    # 16 key/n tiles
KT = C // P                    # 6 c_in tiles
CH = 512                       # q chunk (max fp32 psum-bank free dim)
QCH = N // CH                  # 4 chunks
VW = 3 * 65                    # v width: 3 heads x (64 + fused ones column)
NG = 4                         # groups per block (4 key-tiles per group)
USE_PE_BCAST = True            # den-broadcast via ones-row matmul (gpsimd chain stalls the pipeline)

f32 = mybir.dt.float32
bf16 = mybir.dt.bfloat16
EXP = mybir.ActivationFunctionType.Exp


def _emit(tc, nc, xT, w_qk, w_v, b_qk, w_p, vones, onesrow, y):
    from contextlib import ExitStack

    with ExitStack() as ctx:
        consts = ctx.enter_context(tc.tile_pool(name="consts", bufs=1))
        qk_pool = ctx.enter_context(tc.tile_pool(name="qk", bufs=HPC))
        qk2_pool = ctx.enter_context(tc.tile_pool(name="qk2", bufs=HPC))
        v_pool = ctx.enter_context(tc.tile_pool(name="v", bufs=NT))
        ctx_pool = ctx.enter_context(tc.tile_pool(name="ctxp", bufs=HPC))
        y_pool = ctx.enter_context(tc.tile_pool(name="y", bufs=3))
        r_pool = ctx.enter_context(tc.tile_pool(name="r", bufs=2))
        r0_pool = ctx.enter_context(tc.tile_pool(name="r0", bufs=2))
        bc_pool = ctx.enter_context(tc.tile_pool(name="bc", bufs=2))
        p_pool = ctx.enter_context(tc.tile_pool(name="p", bufs=6))
        x_pool = ctx.enter_context(tc.tile_pool(name="x", bufs=KT))
        ps_s = ctx.enter_context(tc.tile_pool(name="ps_s", bufs=2, space="PSUM"))
        ps_c = ctx.enter_context(tc.tile_pool(name="ps_c", bufs=2, space="PSUM"))
        ps_a = ctx.enter_context(tc.tile_pool(name="ps_a", bufs=2, space="PSUM"))

        # ---- PE warm-up + exp table preload on a memset tile (no DMA dep).
        # The HAM clock gate needs ~3.4us of sustained matmul activity to lift
        # the PE from 1.2 to 2.4 GHz; spin while the x/weight DMAs land.
        warm_sb = consts.tile([P, 256], bf16)
        nc.vector.memset(warm_sb[:], 0.0)
        wps = ps_a.tile([P, CH], f32, tag="ps_a", name="warm_ps")
        for _ in range(95):
            nc.tensor.matmul(
                wps[:, 0:HD], warm_sb[:, 0:P], warm_sb[:, 0:HD], start=True, stop=True
            )
        actwarm = consts.tile([P, 256], bf16)
        nc.scalar.activation(actwarm[:], warm_sb[:], EXP)

        # ---- tiles for constants (DMAs emitted below in priority order)
        vones_sb = consts.tile([P, VW], bf16)
        wqk_sb = consts.tile([P, KT, 2 * HD * HPC], bf16)
        _wqk = w_qk.rearrange("(t p) m -> p t m", p=P)
        wv_sb = consts.tile([P, KT, VW], bf16)
        bqk_sb = consts.tile([P, HPC], f32)
        wp_sb = consts.tile([HD, HPC, C], bf16)
        onesrow_sb = consts.tile([HD + 1, HD], bf16)

        # persistent activations
        qk_sb = [qk_pool.tile([P, N], bf16, tag="qk", name=f"qk{_}") for _ in range(HPC)]
        qk2_sb = [qk2_pool.tile([P, N], bf16, tag="qk2", name=f"qk2_{_}") for _ in range(HPC)]
        v_sb = [v_pool.tile([P, VW], bf16, tag="v", name=f"v{_}") for _ in range(NT)]
        ctx_sb = [ctx_pool.tile([HD, N], bf16, tag="ctx", name=f"ctx{_}") for _ in range(HPC)]
        x_sb = [x_pool.tile([P, N], bf16, tag="x", name=f"x{_}") for _ in range(KT)]

        # x in column-waves: all 6 kt tiles of q-chunk w land together, so the
        # chunk-w qk projection (and v tiles 4w..4w+3) can start before the
        # rest of x arrives.  DMA priority: wave 0 + qkv weights first (they
        # gate the first compute), the late-needed wp/onesrow last.
        def x_wave(w):
            for kt in range(KT):
                nc.sync.dma_start(
                    x_sb[kt][:, w * CH : (w + 1) * CH],
                    xT[kt * P : (kt + 1) * P, w * CH : (w + 1) * CH],
                )

        x_wave(0)
        for kt in range(KT):
            nc.sync.dma_start(wqk_sb[:, kt, :], _wqk[:, kt, :])
        nc.sync.dma_start(wv_sb[:], w_v.rearrange("(t p) m -> p t m", p=P))
        nc.sync.dma_start(vones_sb[:], vones[:])
        nc.sync.dma_start(bqk_sb[:], b_qk.rearrange("t p -> p t"))

        # ---------- unit emitters ----------
        def emit_qk_group(t, cc):
            # qk^T head tile t, q-chunk cc: [q^T(64); k^T(64)] x CH
            sl = slice(cc * CH, (cc + 1) * CH)
            ps = ps_a.tile([P, CH], f32, tag="ps_a", name="ps_qk")
            for kt in range(KT):
                nc.tensor.matmul(
                    ps[:],
                    wqk_sb[:, kt, t * P : (t + 1) * P],
                    x_sb[kt][:, sl],
                    start=(kt == 0),
                    stop=(kt == KT - 1),
                )
            nc.vector.tensor_scalar_add(qk_sb[t][:, sl], ps[:], bqk_sb[:, t : t + 1])
            # swapped copy (k^T to partitions 0:64, q^T to 64:128) so S-pairs
            # can target opposite PE row groups
            nc.sync.dma_start(qk2_sb[t][0:HD, sl], qk_sb[t][HD:P, sl])
            nc.sync.dma_start(qk2_sb[t][HD:P, sl], qk_sb[t][0:HD, sl])

        def emit_v(nt):
            # v natural layout [key, 3*65] (+ ones columns)
            ps = ps_a.tile([P, CH], f32, tag="ps_a", name="ps_v")
            for kt in range(KT):
                nc.tensor.matmul(
                    ps[:, 0:VW],
                    x_sb[kt][:, nt * P : (nt + 1) * P],
                    wv_sb[:, kt, :],
                    start=(kt == 0),
                    stop=(kt == KT - 1),
                )
            nc.vector.tensor_add(v_sb[nt][:], ps[:, 0:VW], vones_sb[:])

        def emit_S_pair(sps, c, h, kp):
            # S^T for key tiles (2kp, 2kp+1) into opposite PE row groups
            kt0, kt1 = 2 * kp, 2 * kp + 1
            nc.tensor.matmul(
                sps[:, 0:CH],
                qk2_sb[h][0:HD, kt0 * P : (kt0 + 1) * P],
                qk_sb[h][0:HD, c * CH : (c + 1) * CH],
            )
            nc.tensor.matmul(
                sps[:, CH : 2 * CH],
                qk_sb[h][HD:P, kt1 * P : (kt1 + 1) * P],
                qk2_sb[h][HD:P, c * CH : (c + 1) * CH],
            )

        def emit_PV(cps, pt, h, kp, first, last):
            kt0, kt1 = 2 * kp, 2 * kp + 1
            nc.tensor.matmul(
                cps[:],
                v_sb[kt0][:, h * 65 : (h + 1) * 65],
                pt[:, 0:CH],
                start=first,
                stop=False,
            )
            nc.tensor.matmul(
                cps[:],
                v_sb[kt1][:, h * 65 : (h + 1) * 65],
                pt[:, CH : 2 * CH],
                start=False,
                stop=last,
            )

        # E projection: per n-tile, psA covers y columns 0:512, psB 512:768.
        ysb_map = {}

        def emit_E_A(nt):
            psA = ps_a.tile([P, CH], f32, tag="ps_a", name="psA")
            for h in range(HPC):
                nc.tensor.matmul(
                    psA[:],
                    ctx_sb[h][:, nt * P : (nt + 1) * P],
                    wp_sb[:, h, 0:CH],
                    start=(h == 0),
                    stop=(h == HPC - 1),
                )
            ysb = y_pool.tile([P, C], bf16, tag="y", name="ysb")
            ysb_map[nt] = ysb
            nc.vector.tensor_copy(ysb[:, 0:CH], psA[:])

        def emit_E_B(nt):
            psB = ps_a.tile([P, CH], f32, tag="ps_a", name="psB")
            for h in range(HPC):
                nc.tensor.matmul(
                    psB[:, 0 : C - CH],
                    ctx_sb[h][:, nt * P : (nt + 1) * P],
                    wp_sb[:, h, CH:C],
                    start=(h == 0),
                    stop=(h == HPC - 1),
                )
            ysb = ysb_map.pop(nt)
            nc.vector.tensor_copy(ysb[:, CH:C], psB[:, 0 : C - CH])
            nc.sync.dma_start(y[nt * P : (nt + 1) * P, :], ysb[:])

        # D normalize, staged: (1) reciprocal of the den row (DVE, PSUM->SBUF),
        # shift it to partition 0 (tiny DMA), broadcast across 64 partitions
        # (GPSIMD); (2) one tensor_mul cps * bc -> ctx (DVE).
        def emit_D1_gpsimd(c, h, cps):
            # den row PSUM->SBUF (f32), reciprocal on SBUF, shift to partition
            # 0 with a tiny DMA, then broadcast on the idle GPSIMD engine.
            dn = r_pool.tile([P, CH], f32, tag="r", name="dn")
            nc.vector.tensor_copy(dn[64:65, :], cps[64:65, :])
            r0 = r0_pool.tile([1, CH], f32, tag="r0", name="r0")
            nc.sync.dma_start(r0[0:1, :], dn[64:65, :])
            r1 = r0_pool.tile([1, CH], f32, tag="r0", name="r1")
            nc.vector.reciprocal_approx_fast(r1[0:1, :], r0[0:1, :])
            bc = bc_pool.tile([HD, CH], f32, tag="bc", name="bc")
            nc.gpsimd.partition_broadcast(bc[:], r1[0:1, :], channels=HD)
            return bc

        # v1-style fallback: den broadcast across partitions via a ones-row
        # matmul, then reciprocal on the broadcast copy.
        def emit_D1_pe(c, h, cps):
            denr = r_pool.tile([P, CH], bf16, tag="r", name="denr")
            nc.vector.tensor_copy(denr[64:65, :], cps[64:65, :])
            bps = ps_a.tile([P, CH], f32, tag="ps_a", name="bps")
            nc.tensor.matmul(
                bps[0:HD, :], onesrow_sb[HD : HD + 1, :], denr[64:65, :],
                start=True, stop=True,
            )
            bcd = bc_pool.tile([HD, CH], f32, tag="bc", name="bcd")
            nc.vector.tensor_copy(bcd[:], bps[0:HD, :])
            bc = bc_pool.tile([HD, CH], f32, tag="bc", name="bc")
            nc.vector.reciprocal_approx_fast(bc[:], bcd[:])
            return bc

        emit_D1 = emit_D1_pe if USE_PE_BCAST else emit_D1_gpsimd

        def emit_D2(c, h, cps, bc):
            nc.vector.tensor_mul(
                ctx_sb[h][:, c * CH : (c + 1) * CH], cps[0:HD, :], bc[:]
            )

        # ---------- block schedule ----------
        # blocks in (h outer, c inner) order; within a block, 4 groups of
        # 4 key-tiles; PV lags its exp by one group slot.
        blocks = [(h, c) for h in range(HPC) for c in range(QCH)]

        # per-block 128-class work lists (qk units for the next head; v units
        # in the very first block)
        work128 = {bi: [] for bi in range(len(blocks))}
        for bi, (h, c) in enumerate(blocks):
            if h == 0 and c == 0:
                for g in range(NG):
                    work128[bi].append(("v4", g))   # v tiles 4g..4g+3
                work128[bi].append(("qk", 1, 0))    # qk(h1, c0) at block end
            elif h < HPC - 1:
                if not (h == 0 and c == 0):
                    work128[bi].append(("qk", h + 1, c))

        # prologue: qk for head 0, interleaved with the remaining x waves so
        # each chunk's qk2-swap DMA sits ahead of the next wave in queue order
        # (a swap queued behind all of x delays the first S-pairs by ~20us).
        emit_qk_group(0, 0)
        x_wave(1)
        emit_qk_group(0, 1)
        x_wave(2)
        emit_qk_group(0, 2)
        x_wave(3)
        nc.sync.dma_start(wp_sb[:], w_p.rearrange("(h p) m -> p h m", p=HD))
        nc.sync.dma_start(onesrow_sb[:], onesrow[:])
        emit_qk_group(0, 3)

        pend_pv = None          # (ptA, ptB, h, c, g) awaiting PV in next slot
        pend_D = []             # staged D work: dicts
        pend_E = []             # E half closures ready to emit in 64-slots
        cps_cur = [None]        # cps tile of the block being accumulated

        def emit_pv_slot():
            # PVs for the lagged group (4 matmuls, 2 pt tiles)
            if pend_pv is None:
                return
            ptA, ptB, h, c, g = pend_pv
            if g == 0:
                cps_cur[0] = ps_c.tile([65, CH], f32, tag="ps_c", name="cps")
            cps = cps_cur[0]
            emit_PV(cps, ptA, h, 2 * g, first=(g == 0), last=False)
            emit_PV(cps, ptB, h, 2 * g + 1, first=False, last=(g == NG - 1))
            if g == NG - 1:
                pend_D.append({"c": c, "h": h, "cps": cps, "bc": None, "stage": 0})

        def run_D_stage():
            if not pend_D:
                return
            d = pend_D[0]
            if d["stage"] == 0:
                d["bc"] = emit_D1(d["c"], d["h"], d["cps"])
                d["stage"] = 1
            else:
                emit_D2(d["c"], d["h"], d["cps"], d["bc"])
                pend_D.pop(0)
                if d["h"] == HPC - 1:
                    cc = d["c"]
                    for i in range(CH // P):
                        nt = cc * (CH // P) + i
                        pend_E.append(("A", nt))
                        pend_E.append(("B", nt))

        def flush_E(budget):
            while budget > 0 and pend_E:
                kind, nt = pend_E.pop(0)
                if kind == "A":
                    emit_E_A(nt)
                else:
                    emit_E_B(nt)
                budget -= 1

        for bi, (h, c) in enumerate(blocks):
            wq = list(work128[bi])
            for g in range(NG):
                # --- (64,128)-class slot: 2 S-pairs + E units
                spsA = ps_s.tile([P, 2 * CH], f32, tag="ps_s", name="spsA")
                emit_S_pair(spsA, c, h, 2 * g)
                spsB = ps_s.tile([P, 2 * CH], f32, tag="ps_s", name="spsB")
                emit_S_pair(spsB, c, h, 2 * g + 1)
                flush_E(2)
                # --- ScalarE: exp of both halves
                ptA = p_pool.tile([P, 2 * CH], bf16, tag="p", name="ptA")
                nc.scalar.activation(ptA[:], spsA[:], EXP)
                ptB = p_pool.tile([P, 2 * CH], bf16, tag="p", name="ptB")
                nc.scalar.activation(ptB[:], spsB[:], EXP)
                # --- D chain stages (DVE/GPSIMD/DMA only)
                run_D_stage()
                # --- (128,*)-class slot: qk unit / v units
                if wq:
                    kind = wq[0]
                    if kind[0] == "v4":
                        gg = kind[1]
                        for nt in range(4 * gg, 4 * gg + 4):
                            emit_v(nt)
                        wq.pop(0)
                    elif kind[0] == "qk" and g >= NG - 2:
                        # emit the qk unit late in the block so its 6-matmul
                        # burst lands after the block's own S supply is ahead
                        emit_qk_group(kind[1], kind[2])
                        wq.pop(0)
                # --- (128,65)-class slot: lagged PVs
                emit_pv_slot()
                pend_pv = (ptA, ptB, h, c, g)
            # any 128-work not emitted (shouldn't happen): emit now
            for kind in wq:
                if kind[0] == "v4":
                    for nt in range(4 * kind[1], 4 * kind[1] + 4):
                        emit_v(nt)
                else:
                    emit_qk_group(kind[1], kind[2])

        # ---------- drain ----------
        emit_pv_slot()
        pend_pv = None
        while pend_D:
            run_D_stage()
        flush_E(len(pend_E))


def build_program():
    nc = bacc.Bacc("TRN2", target_bir_lowering=False, debug=False)
    xT = nc.dram_tensor("xT", [C, N], bf16, kind="ExternalInput").ap()
    w_qk = nc.dram_tensor("w_qk", [C, 2 * HD * HPC], bf16, kind="ExternalInput").ap()
    w_v = nc.dram_tensor("w_v", [C, VW], bf16, kind="ExternalInput").ap()
    b_qk = nc.dram_tensor("b_qk", [HPC, P], f32, kind="ExternalInput").ap()
    w_p = nc.dram_tensor("w_p", [HPC * HD, C], bf16, kind="ExternalInput").ap()
    vones = nc.dram_tensor("vones", [P, VW], bf16, kind="ExternalInput").ap()
    onesrow = nc.dram_tensor("onesrow", [HD + 1, HD], bf16, kind="ExternalInput").ap()
    y = nc.dram_tensor("y", [N, C], bf16, kind="ExternalOutput").ap()
    with tile.TileContext(nc) as tc:
        _emit(tc, nc, xT, w_qk, w_v, b_qk, w_p, vones, onesrow, y)
    nc.compile()
    return nc


_CACHE = {}


def _get_program():
    if "nc" not in _CACHE:
        _CACHE["nc"] = build_program()
    return _CACHE["nc"]


def make_in_maps(x, W_qkv, b_qkv, W_proj):
    """Per-core input dicts implementing the (batch, head-group) sharding."""
    x = np.ascontiguousarray(np.asarray(x, np.float32))
    W_qkv = np.asarray(W_qkv, np.float32)
    b_qkv = np.asarray(b_qkv, np.float32)
    W_proj = np.asarray(W_proj, np.float32)
    scale = float(HD) ** -0.5

    Wq = W_qkv[0:C].reshape(H, HD, C)
    Wk = W_qkv[C : 2 * C].reshape(H, HD, C)
    Wv = W_qkv[2 * C : 3 * C].reshape(H, HD, C)
    bq = b_qkv[0:C].reshape(H, HD)
    bk = b_qkv[C : 2 * C].reshape(H, HD)

    vones_mask = np.zeros((P, VW), np.float32)
    for i in range(HPC):
        vones_mask[:, i * 65 + HD] = 1.0
    onesrow_arr = np.zeros((HD + 1, HD), np.float32)
    onesrow_arr[HD, :] = 1.0

    in_maps = []
    for core in range(NCORES):
        b = core // CORES_PER_B
        hg = core % CORES_PER_B
        heads = list(range(hg * HPC, (hg + 1) * HPC))

        xT = np.ascontiguousarray(x[b].T).astype(ml_dtypes.bfloat16)  # [C, N]
        w_qk = np.empty((C, 2 * HD * HPC), np.float32)  # cast to bf16 below
        b_qk_arr = np.empty((HPC, P), np.float32)
        w_v = np.zeros((C, VW), np.float32)
        w_p = np.empty((HPC * HD, C), np.float32)
        for i, h in enumerate(heads):
            w_qk[:, i * P : i * P + HD] = Wq[h].T * scale
            w_qk[:, i * P + HD : (i + 1) * P] = Wk[h].T
            b_qk_arr[i, 0:HD] = bq[h] * scale
            b_qk_arr[i, HD:P] = bk[h]
            w_v[:, i * 65 : i * 65 + HD] = Wv[h].T
            w_p[i * HD : (i + 1) * HD, :] = W_proj[:, h * HD : (h + 1) * HD].T
        in_maps.append(
            {"xT": xT,
             "w_qk": w_qk.astype(ml_dtypes.bfloat16),
             "w_v": w_v.astype(ml_dtypes.bfloat16),
             "b_qk": b_qk_arr,
             "w_p": w_p.astype(ml_dtypes.bfloat16),
             "vones": vones_mask.astype(ml_dtypes.bfloat16),
             "onesrow": onesrow_arr.astype(ml_dtypes.bfloat16)}
        )
    return in_maps


def gather_output(results, b_qkv, W_proj, b_proj):
    """Sum the per-core partial projections (TP all-reduce) + effective bias."""
    out = np.zeros((B, N, C), np.float32)
    for core in range(NCORES):
        out[core // CORES_PER_B] += np.asarray(results[core]["y"], np.float32)
    b_v = np.asarray(b_qkv, np.float32)[2 * C : 3 * C]
    b_eff = np.asarray(b_proj, np.float32) + np.asarray(W_proj, np.float32) @ b_v
    out += b_eff
    return out


def kernel(x=None, xpos=None, W_qkv=None, b_qkv=None, W_proj=None, b_proj=None, **kw):
    del xpos, kw  # rope disabled in this configuration; xpos unused
    nc = _get_program()
    in_maps = make_in_maps(x, W_qkv, b_qkv, W_proj)
    res = run_bass_kernel_spmd(nc, in_maps, core_ids=list(range(NCORES)))
    return gather_output(res.results, b_qkv, W_proj, b_proj)


# revision 11
# speedup vs baseline: 1.1590x; 1.1590x over previous
"""Trainium2 Bass kernel for a 12-head attention block (B=2, N=2048, C=768).

Sharding: the 24 (batch, head) pairs are split across 8 NeuronCores —
4 cores per batch element, 3 heads per core (data + head/tensor parallel).
Each core computes qkv projections for its heads, the full attention for
its heads (the N x N score matrix is private to a core), and a *partial*
output projection over its heads' channels.  The host sums the 4 partial
projections per batch element (the tensor-parallel all-reduce) and adds
the bias.

Device algorithm (activations/weights bf16, fp32 PSUM accumulation):

  xT [768, 2048] (x transposed on host)
  B:  qk^T  = W_qk^T.T @ xT  -> per-head tile [q^T(64 rows); k^T(64)] x 2048
      (attention scale 1/8 and b_q, b_k folded into W/bias on host)
  B2: v     = xT.T @ W_v^T   -> [2048, 3*65] with a column of ones per head
  C:  S^T[key, q] = k^T.T @ q^T        (per 128-key tile, 512-q chunk)
      P^T = exp(S^T)                   (ScalarE, no max subtraction:
                                        logits are in [-3, 3] by construction)
      ctx_u^T[d|den, q] += [v | 1].T @ P^T   (fused denominator row)
  D:  ctx^T = ctx_u^T[0:64] * (1/den)  (reciprocal on DVE, den row shifted to
      partition 0 by a tiny SBUF DMA, broadcast across partitions on the
      otherwise-idle GPSIMD engine, then one tensor_mul)
  E:  y[n, :] += ctx^T.T @ W_p^T      (partial projection, summed on host)

Scheduling notes: PE tile-config switches (64x128 <-> 128x128 <-> 128x65)
cost ~107ns of array drain each, so matmuls are emitted in same-shape
groups covering two key-tile pairs at a time: [S x4][proj/E][qk or v][PV x4],
with the PV consumers lagging one group behind their exp.  The PE warm-up
spin runs on a memset tile so it needs no DMA and starts immediately; the
exp table set is preloaded the same way.  x arrives in four column-waves so
the first qk projection (and the first S-pairs) start ~3us in.
"""

import numpy as np
import ml_dtypes

import concourse.bass as bass
import concourse.bacc as bacc
import concourse.tile as tile
import concourse.mybir as mybir
from concourse.bass_utils import run_bass_kernel_spmd

# Problem shape (hardcoded; harness contract)
B, N, C = 2, 2048, 768
H, HD = 12, 64
NCORES = 8
CORES_PER_B = NCORES // B      # 4
HPC = H // CORES_PER_B         # 3 heads per core
P = 128
NT = N // P                    # 16 key/n tiles
KT = C // P                    # 6 c_in tiles
CH = 512                       # q chunk (max fp32 psum-bank free dim)
QCH = N // CH                  # 4 chunks
VW = 3 * 65                    # v width: 3 heads x (64 + fused ones column)
NG = 4                         # groups per block (4 key-tiles per group)
USE_PE_BCAST = True            # den-broadcast via ones-row matmul (gpsimd chain stalls the pipeline)

f32 = mybir.dt.float32
bf16 = mybir.dt.bfloat16
EXP = mybir.ActivationFunctionType.Exp


def _emit(tc, nc, xT, w_qk, w_v, b_qk, w_p, vones, onesrow, swapm, y):
    from contextlib import ExitStack

    with ExitStack() as ctx:
        consts = ctx.enter_context(tc.tile_pool(name="consts", bufs=1))
        qk_pool = ctx.enter_context(tc.tile_pool(name="qk", bufs=HPC))
        qk2_pool = ctx.enter_context(tc.tile_pool(name="qk2", bufs=HPC))
        v_pool = ctx.enter_context(tc.tile_pool(name="v", bufs=NT))
        ctx_pool = ctx.enter_context(tc.tile_pool(name="ctxp", bufs=HPC))
        y_pool = ctx.enter_context(tc.tile_pool(name="y", bufs=3))
        r_pool = ctx.enter_context(tc.tile_pool(name="r", bufs=2))
        r0_pool = ctx.enter_context(tc.tile_pool(name="r0", bufs=2))
        bc_pool = ctx.enter_context(tc.tile_pool(name="bc", bufs=2))
        p_pool = ctx.enter_context(tc.tile_pool(name="p", bufs=6))
        x_pool = ctx.enter_context(tc.tile_pool(name="x", bufs=KT))
        ps_s = ctx.enter_context(tc.tile_pool(name="ps_s", bufs=2, space="PSUM"))
        ps_c = ctx.enter_context(tc.tile_pool(name="ps_c", bufs=2, space="PSUM"))
        ps_a = ctx.enter_context(tc.tile_pool(name="ps_a", bufs=2, space="PSUM"))

        # ---- PE warm-up + exp table preload on a memset tile (no DMA dep).
        # The HAM clock gate needs ~3.4us of sustained matmul activity to lift
        # the PE from 1.2 to 2.4 GHz; spin while the x/weight DMAs land.
        warm_sb = consts.tile([P, 256], bf16)
        nc.vector.memset(warm_sb[:], 0.0)
        wps = ps_a.tile([P, CH], f32, tag="ps_a", name="warm_ps")
        for _ in range(95):
            nc.tensor.matmul(
                wps[:, 0:HD], warm_sb[:, 0:P], warm_sb[:, 0:HD], start=True, stop=True
            )
        actwarm = consts.tile([P, 256], bf16)
        nc.scalar.activation(actwarm[:], warm_sb[:], EXP)

        # ---- tiles for constants (DMAs emitted below in priority order)
        vones_sb = consts.tile([P, VW], bf16)
        wqk_sb = consts.tile([P, KT, 2 * HD * HPC], bf16)
        _wqk = w_qk.rearrange("(t p) m -> p t m", p=P)
        wv_sb = consts.tile([P, KT, VW], bf16)
        bqk_sb = consts.tile([P, HPC], f32)
        wp_sb = consts.tile([HD, HPC, C], bf16)
        onesrow_sb = consts.tile([HD + 1, HD], bf16)
        swapm_sb = consts.tile([P, P], bf16)

        # persistent activations
        qk_sb = [qk_pool.tile([P, N], bf16, tag="qk", name=f"qk{_}") for _ in range(HPC)]
        qk2_sb = [qk2_pool.tile([P, N], bf16, tag="qk2", name=f"qk2_{_}") for _ in range(HPC)]
        v_sb = [v_pool.tile([P, VW], bf16, tag="v", name=f"v{_}") for _ in range(NT)]
        ctx_sb = [ctx_pool.tile([HD, N], bf16, tag="ctx", name=f"ctx{_}") for _ in range(HPC)]
        x_sb = [x_pool.tile([P, N], bf16, tag="x", name=f"x{_}") for _ in range(KT)]

        # x in column-waves: all 6 kt tiles of q-chunk w land together, so the
        # chunk-w qk projection (and v tiles 4w..4w+3) can start before the
        # rest of x arrives.  DMA priority: wave 0 + qkv weights first (they
        # gate the first compute), the late-needed wp/onesrow last.
        def x_wave(w):
            for kt in range(KT):
                nc.sync.dma_start(
                    x_sb[kt][:, w * CH : (w + 1) * CH],
                    xT[kt * P : (kt + 1) * P, w * CH : (w + 1) * CH],
                )

        x_wave(0)
        for kt in range(KT):
            nc.sync.dma_start(wqk_sb[:, kt, :], _wqk[:, kt, :])
        nc.sync.dma_start(wv_sb[:], w_v.rearrange("(t p) m -> p t m", p=P))
        nc.sync.dma_start(vones_sb[:], vones[:])
        nc.sync.dma_start(bqk_sb[:], b_qk.rearrange("t p -> p t"))
        nc.sync.dma_start(swapm_sb[:], swapm[:])

        # ---------- unit emitters ----------
        def emit_qk_group(t, cc, pe_swap=False):
            # qk^T head tile t, q-chunk cc: [q^T(64); k^T(64)] x CH
            sl = slice(cc * CH, (cc + 1) * CH)
            ps = ps_a.tile([P, CH], f32, tag="ps_a", name="ps_qk")
            for kt in range(KT):
                nc.tensor.matmul(
                    ps[:],
                    wqk_sb[:, kt, t * P : (t + 1) * P],
                    x_sb[kt][:, sl],
                    start=(kt == 0),
                    stop=(kt == KT - 1),
                )
            nc.vector.tensor_scalar_add(qk_sb[t][:, sl], ps[:], bqk_sb[:, t : t + 1])
            # swapped copy (k^T to partitions 0:64, q^T to 64:128) so S-pairs
            # can target opposite PE row groups
            if pe_swap:
                sps2 = ps_a.tile([P, CH], f32, tag="ps_a", name="ps_swap")
                nc.tensor.matmul(sps2[:], swapm_sb[:], qk_sb[t][:, sl],
                                 start=True, stop=True)
                nc.vector.tensor_copy(qk2_sb[t][:, sl], sps2[:])
            else:
                nc.sync.dma_start(qk2_sb[t][0:HD, sl], qk_sb[t][HD:P, sl])
                nc.sync.dma_start(qk2_sb[t][HD:P, sl], qk_sb[t][0:HD, sl])

        def emit_v(nt):
            # v natural layout [key, 3*65] (+ ones columns)
            ps = ps_a.tile([P, CH], f32, tag="ps_a", name="ps_v")
            for kt in range(KT):
                nc.tensor.matmul(
                    ps[:, 0:VW],
                    x_sb[kt][:, nt * P : (nt + 1) * P],
                    wv_sb[:, kt, :],
                    start=(kt == 0),
                    stop=(kt == KT - 1),
                )
            nc.vector.tensor_add(v_sb[nt][:], ps[:, 0:VW], vones_sb[:])

        def emit_S_pair(sps, c, h, kp):
            # S^T for key tiles (2kp, 2kp+1) into opposite PE row groups
            kt0, kt1 = 2 * kp, 2 * kp + 1
            nc.tensor.matmul(
                sps[:, 0:CH],
                qk2_sb[h][0:HD, kt0 * P : (kt0 + 1) * P],
                qk_sb[h][0:HD, c * CH : (c + 1) * CH],
            )
            nc.tensor.matmul(
                sps[:, CH : 2 * CH],
                qk_sb[h][HD:P, kt1 * P : (kt1 + 1) * P],
                qk2_sb[h][HD:P, c * CH : (c + 1) * CH],
            )

        def emit_PV(cps, pt, h, kp, first, last):
            kt0, kt1 = 2 * kp, 2 * kp + 1
            nc.tensor.matmul(
                cps[:],
                v_sb[kt0][:, h * 65 : (h + 1) * 65],
                pt[:, 0:CH],
                start=first,
                stop=False,
            )
            nc.tensor.matmul(
                cps[:],
                v_sb[kt1][:, h * 65 : (h + 1) * 65],
                pt[:, CH : 2 * CH],
                start=False,
                stop=last,
            )

        # E projection: per n-tile, psA covers y columns 0:512, psB 512:768.
        ysb_map = {}

        def emit_E_A(nt):
            psA = ps_a.tile([P, CH], f32, tag="ps_a", name="psA")
            for h in range(HPC):
                nc.tensor.matmul(
                    psA[:],
                    ctx_sb[h][:, nt * P : (nt + 1) * P],
                    wp_sb[:, h, 0:CH],
                    start=(h == 0),
                    stop=(h == HPC - 1),
                )
            ysb = y_pool.tile([P, C], bf16, tag="y", name="ysb")
            ysb_map[nt] = ysb
            nc.vector.tensor_copy(ysb[:, 0:CH], psA[:])

        def emit_E_B(nt):
            psB = ps_a.tile([P, CH], f32, tag="ps_a", name="psB")
            for h in range(HPC):
                nc.tensor.matmul(
                    psB[:, 0 : C - CH],
                    ctx_sb[h][:, nt * P : (nt + 1) * P],
                    wp_sb[:, h, CH:C],
                    start=(h == 0),
                    stop=(h == HPC - 1),
                )
            ysb = ysb_map.pop(nt)
            nc.vector.tensor_copy(ysb[:, CH:C], psB[:, 0 : C - CH])
            nc.sync.dma_start(y[nt * P : (nt + 1) * P, :], ysb[:])

        # D normalize, staged: (1) reciprocal of the den row (DVE, PSUM->SBUF),
        # shift it to partition 0 (tiny DMA), broadcast across 64 partitions
        # (GPSIMD); (2) one tensor_mul cps * bc -> ctx (DVE).
        def emit_D1_gpsimd(c, h, cps):
            # den row PSUM->SBUF (f32), reciprocal on SBUF, shift to partition
            # 0 with a tiny DMA, then broadcast on the idle GPSIMD engine.
            dn = r_pool.tile([P, CH], f32, tag="r", name="dn")
            nc.vector.tensor_copy(dn[64:65, :], cps[64:65, :])
            r0 = r0_pool.tile([1, CH], f32, tag="r0", name="r0")
            nc.sync.dma_start(r0[0:1, :], dn[64:65, :])
            r1 = r0_pool.tile([1, CH], f32, tag="r0", name="r1")
            nc.vector.reciprocal_approx_fast(r1[0:1, :], r0[0:1, :])
            bc = bc_pool.tile([HD, CH], f32, tag="bc", name="bc")
            nc.gpsimd.partition_broadcast(bc[:], r1[0:1, :], channels=HD)
            return bc

        # v1-style: den broadcast across partitions via a ones-row matmul,
        # then reciprocal on the broadcast copy.
        def emit_D1_pe(c, h, cps):
            denr = r_pool.tile([P, CH], bf16, tag="r", name="denr")
            nc.vector.tensor_copy(denr[64:65, :], cps[64:65, :])
            bps = ps_a.tile([P, CH], f32, tag="ps_a", name="bps")
            nc.tensor.matmul(
                bps[0:HD, :], onesrow_sb[HD : HD + 1, :], denr[64:65, :],
                start=True, stop=True,
            )
            bcd = bc_pool.tile([HD, CH], f32, tag="bc", name="bcd")
            nc.vector.tensor_copy(bcd[:], bps[0:HD, :])
            bc = bc_pool.tile([HD, CH], f32, tag="bc", name="bc")
            nc.vector.reciprocal_approx_fast(bc[:], bcd[:])
            return bc

        emit_D1 = emit_D1_pe if USE_PE_BCAST else emit_D1_gpsimd

        def emit_D2(c, h, cps, bc):
            nc.vector.tensor_mul(
                ctx_sb[h][:, c * CH : (c + 1) * CH], cps[0:HD, :], bc[:]
            )

        # ---------- block schedule ----------
        # blocks in (h outer, c inner) order; within a block, 4 groups of
        # 4 key-tiles; PV lags its exp by one group slot.
        blocks = [(h, c) for h in range(HPC) for c in range(QCH)]

        # per-block 128-class work lists (qk units for the next head; v units
        # in the very first block)
        work128 = {bi: [] for bi in range(len(blocks))}
        for bi, (h, c) in enumerate(blocks):
            if h == 0 and c == 0:
                for g in range(NG):
                    work128[bi].append(("v4", g))   # v tiles 4g..4g+3
                work128[bi].append(("qk", 1, 0))    # qk(h1, c0) at block end
            elif h < HPC - 1:
                if not (h == 0 and c == 0):
                    work128[bi].append(("qk", h + 1, c))

        x_wave(1)
        x_wave(2)
        x_wave(3)
        nc.sync.dma_start(wp_sb[:], w_p.rearrange("(h p) m -> p h m", p=HD))
        nc.sync.dma_start(onesrow_sb[:], onesrow[:])
        # prologue: qk for head 0, all chunks.  The qk2 swap for head 0 runs
        # on PE+DVE (permutation matmul) — a swap DMA would queue behind the
        # x waves and delay the first S-pairs by ~20us.
        for cc in range(QCH):
            emit_qk_group(0, cc, pe_swap=True)

        pend_pv = None          # (ptA, ptB, h, c, g) awaiting PV in next slot
        pend_D = []             # staged D work: dicts
        pend_E = []             # E half closures ready to emit in 64-slots
        cps_cur = [None]        # cps tile of the block being accumulated

        def emit_pv_slot():
            # PVs for the lagged group (4 matmuls, 2 pt tiles)
            if pend_pv is None:
                return
            ptA, ptB, h, c, g = pend_pv
            if g == 0:
                cps_cur[0] = ps_c.tile([65, CH], f32, tag="ps_c", name="cps")
            cps = cps_cur[0]
            emit_PV(cps, ptA, h, 2 * g, first=(g == 0), last=False)
            emit_PV(cps, ptB, h, 2 * g + 1, first=False, last=(g == NG - 1))
            if g == NG - 1:
                pend_D.append({"c": c, "h": h, "cps": cps, "bc": None, "stage": 0})

        def run_D_stage():
            if not pend_D:
                return
            d = pend_D[0]
            if d["stage"] == 0:
                d["bc"] = emit_D1(d["c"], d["h"], d["cps"])
                d["stage"] = 1
            else:
                emit_D2(d["c"], d["h"], d["cps"], d["bc"])
                pend_D.pop(0)
                if d["h"] == HPC - 1:
                    cc = d["c"]
                    for i in range(CH // P):
                        nt = cc * (CH // P) + i
                        pend_E.append(("A", nt))
                        pend_E.append(("B", nt))

        def flush_E(budget):
            while budget > 0 and pend_E:
                kind, nt = pend_E.pop(0)
                if kind == "A":
                    emit_E_A(nt)
                else:
                    emit_E_B(nt)
                budget -= 1

        for bi, (h, c) in enumerate(blocks):
            wq = list(work128[bi])
            for g in range(NG):
                # --- (64,128)-class slot: 2 S-pairs + E units
                spsA = ps_s.tile([P, 2 * CH], f32, tag="ps_s", name="spsA")
                emit_S_pair(spsA, c, h, 2 * g)
                spsB = ps_s.tile([P, 2 * CH], f32, tag="ps_s", name="spsB")
                emit_S_pair(spsB, c, h, 2 * g + 1)
                flush_E(2)
                # --- ScalarE: exp of both halves
                ptA = p_pool.tile([P, 2 * CH], bf16, tag="p", name="ptA")
                nc.scalar.activation(ptA[:], spsA[:], EXP)
                ptB = p_pool.tile([P, 2 * CH], bf16, tag="p", name="ptB")
                nc.scalar.activation(ptB[:], spsB[:], EXP)
                # --- D chain stages (DVE/GPSIMD/DMA only)
                run_D_stage()
                # --- (128,*)-class slot: qk unit / v units
                if wq:
                    kind = wq[0]
                    if kind[0] == "v4":
                        gg = kind[1]
                        for nt in range(4 * gg, 4 * gg + 4):
                            emit_v(nt)
                        wq.pop(0)
                    elif kind[0] == "qk" and g >= NG - 2:
                        # emit the qk unit late in the block so its 6-matmul
                        # burst lands after the block's own S supply is ahead
                        emit_qk_group(kind[1], kind[2])
                        wq.pop(0)
                # --- (128,65)-class slot: lagged PVs
                emit_pv_slot()
                pend_pv = (ptA, ptB, h, c, g)
            # any 128-work not emitted (shouldn't happen): emit now
            for kind in wq:
                if kind[0] == "v4":
                    for nt in range(4 * kind[1], 4 * kind[1] + 4):
                        emit_v(nt)
                else:
                    emit_qk_group(kind[1], kind[2])

        # ---------- drain ----------
        emit_pv_slot()
        pend_pv = None
        while pend_D:
            run_D_stage()
        flush_E(len(pend_E))


def build_program():
    nc = bacc.Bacc("TRN2", target_bir_lowering=False, debug=False)
    xT = nc.dram_tensor("xT", [C, N], bf16, kind="ExternalInput").ap()
    w_qk = nc.dram_tensor("w_qk", [C, 2 * HD * HPC], bf16, kind="ExternalInput").ap()
    w_v = nc.dram_tensor("w_v", [C, VW], bf16, kind="ExternalInput").ap()
    b_qk = nc.dram_tensor("b_qk", [HPC, P], f32, kind="ExternalInput").ap()
    w_p = nc.dram_tensor("w_p", [HPC * HD, C], bf16, kind="ExternalInput").ap()
    vones = nc.dram_tensor("vones", [P, VW], bf16, kind="ExternalInput").ap()
    onesrow = nc.dram_tensor("onesrow", [HD + 1, HD], bf16, kind="ExternalInput").ap()
    swapm = nc.dram_tensor("swapm", [P, P], bf16, kind="ExternalInput").ap()
    y = nc.dram_tensor("y", [N, C], bf16, kind="ExternalOutput").ap()
    with tile.TileContext(nc) as tc:
        _emit(tc, nc, xT, w_qk, w_v, b_qk, w_p, vones, onesrow, swapm, y)
    nc.compile()
    return nc


_CACHE = {}


def _get_program():
    if "nc" not in _CACHE:
        _CACHE["nc"] = build_program()
    return _CACHE["nc"]


def make_in_maps(x, W_qkv, b_qkv, W_proj):
    """Per-core input dicts implementing the (batch, head-group) sharding."""
    x = np.ascontiguousarray(np.asarray(x, np.float32))
    W_qkv = np.asarray(W_qkv, np.float32)
    b_qkv = np.asarray(b_qkv, np.float32)
    W_proj = np.asarray(W_proj, np.float32)
    scale = float(HD) ** -0.5

    Wq = W_qkv[0:C].reshape(H, HD, C)
    Wk = W_qkv[C : 2 * C].reshape(H, HD, C)
    Wv = W_qkv[2 * C : 3 * C].reshape(H, HD, C)
    bq = b_qkv[0:C].reshape(H, HD)
    bk = b_qkv[C : 2 * C].reshape(H, HD)

    vones_mask = np.zeros((P, VW), np.float32)
    for i in range(HPC):
        vones_mask[:, i * 65 + HD] = 1.0
    onesrow_arr = np.zeros((HD + 1, HD), np.float32)
    onesrow_arr[HD, :] = 1.0
    swapm_arr = np.zeros((P, P), np.float32)
    for i in range(P):
        swapm_arr[(i + HD) % P, i] = 1.0

    in_maps = []
    for core in range(NCORES):
        b = core // CORES_PER_B
        hg = core % CORES_PER_B
        heads = list(range(hg * HPC, (hg + 1) * HPC))

        xT = np.ascontiguousarray(x[b].T).astype(ml_dtypes.bfloat16)  # [C, N]
        w_qk = np.empty((C, 2 * HD * HPC), np.float32)  # cast to bf16 below
        b_qk_arr = np.empty((HPC, P), np.float32)
        w_v = np.zeros((C, VW), np.float32)
        w_p = np.empty((HPC * HD, C), np.float32)
        for i, h in enumerate(heads):
            w_qk[:, i * P : i * P + HD] = Wq[h].T * scale
            w_qk[:, i * P + HD : (i + 1) * P] = Wk[h].T
            b_qk_arr[i, 0:HD] = bq[h] * scale
            b_qk_arr[i, HD:P] = bk[h]
            w_v[:, i * 65 : i * 65 + HD] = Wv[h].T
            w_p[i * HD : (i + 1) * HD, :] = W_proj[:, h * HD : (h + 1) * HD].T
        in_maps.append(
            {"xT": xT,
             "w_qk": w_qk.astype(ml_dtypes.bfloat16),
             "w_v": w_v.astype(ml_dtypes.bfloat16),
             "b_qk": b_qk_arr,
             "w_p": w_p.astype(ml_dtypes.bfloat16),
             "vones": vones_mask.astype(ml_dtypes.bfloat16),
             "onesrow": onesrow_arr.astype(ml_dtypes.bfloat16),
             "swapm": swapm_arr.astype(ml_dtypes.bfloat16)}
        )
    return in_maps


def gather_output(results, b_qkv, W_proj, b_proj):
    """Sum the per-core partial projections (TP all-reduce) + effective bias."""
    out = np.zeros((B, N, C), np.float32)
    for core in range(NCORES):
        out[core // CORES_PER_B] += np.asarray(results[core]["y"], np.float32)
    b_v = np.asarray(b_qkv, np.float32)[2 * C : 3 * C]
    b_eff = np.asarray(b_proj, np.float32) + np.asarray(W_proj, np.float32) @ b_v
    out += b_eff
    return out


def kernel(x=None, xpos=None, W_qkv=None, b_qkv=None, W_proj=None, b_proj=None, **kw):
    del xpos, kw  # rope disabled in this configuration; xpos unused
    nc = _get_program()
    in_maps = make_in_maps(x, W_qkv, b_qkv, W_proj)
    res = run_bass_kernel_spmd(nc, in_maps, core_ids=list(range(NCORES)))
    return gather_output(res.results, b_qkv, W_proj, b_proj)


# revision 13
# speedup vs baseline: 1.2186x; 1.0514x over previous
"""Trainium2 Bass kernel for a 12-head attention block (B=2, N=2048, C=768).

Sharding: the 24 (batch, head) pairs are split across 8 NeuronCores —
4 cores per batch element, 3 heads per core (data + head/tensor parallel).
Each core computes qkv projections for its heads, the full attention for
its heads (the N x N score matrix is private to a core), and a *partial*
output projection over its heads' channels.  The host sums the 4 partial
projections per batch element (the tensor-parallel all-reduce) and adds
the bias.

Device algorithm (activations/weights bf16, fp32 PSUM accumulation):

  xT [768, 2048] (x transposed on host)
  B:  qk^T  = W_qk^T.T @ xT  -> per-head tile [q^T(64 rows); k^T(64)] x 2048
      (attention scale 1/8 and b_q, b_k folded into W/bias on host)
  B2: v     = xT.T @ W_v^T   -> [2048, 3*65] with a column of ones per head
  C:  S^T[key, q] = k^T.T @ q^T        (per 128-key tile, 512-q chunk)
      P^T = exp(S^T)                   (ScalarE, no max subtraction:
                                        logits are in [-3, 3] by construction)
      ctx_u^T[d|den, q] += [v | 1].T @ P^T   (fused denominator row)
  D:  ctx^T = ctx_u^T[0:64] * (1/den)  (reciprocal on DVE, den row shifted to
      partition 0 by a tiny SBUF DMA, broadcast across partitions on the
      otherwise-idle GPSIMD engine, then one tensor_mul)
  E:  y[n, :] += ctx^T.T @ W_p^T      (partial projection, summed on host)

Scheduling notes: PE tile-config switches (64x128 <-> 128x128 <-> 128x65)
cost ~107ns of array drain each, so matmuls are emitted in same-shape
groups covering two key-tile pairs at a time: [S x4][proj/E][qk or v][PV x4],
with the PV consumers lagging one group behind their exp.  The PE warm-up
spin runs on a memset tile so it needs no DMA and starts immediately; the
exp table set is preloaded the same way.  x arrives in four column-waves so
the first qk projection (and the first S-pairs) start ~3us in.
"""

import numpy as np
import ml_dtypes

import concourse.bass as bass
import concourse.bacc as bacc
import concourse.tile as tile
import concourse.mybir as mybir
from concourse.bass_utils import run_bass_kernel_spmd

# Problem shape (hardcoded; harness contract)
B, N, C = 2, 2048, 768
H, HD = 12, 64
NCORES = 8
CORES_PER_B = NCORES // B      # 4
HPC = H // CORES_PER_B         # 3 heads per core
P = 128
NT = N // P                    # 16 key/n tiles
KT = C // P                    # 6 c_in tiles
CH = 512                       # q chunk (max fp32 psum-bank free dim)
QCH = N // CH                  # 4 chunks
VW = 3 * 65                    # v width: 3 heads x (64 + fused ones column)
NG = 4                         # groups per block (4 key-tiles per group)
USE_PE_BCAST = True            # den-broadcast via ones-row matmul (gpsimd chain stalls the pipeline)

f32 = mybir.dt.float32
bf16 = mybir.dt.bfloat16
EXP = mybir.ActivationFunctionType.Exp


def _emit(tc, nc, xT, w_qk, w_v, b_qk, w_p, vones, onesrow, swapm, y):
    from contextlib import ExitStack

    with ExitStack() as ctx:
        consts = ctx.enter_context(tc.tile_pool(name="consts", bufs=1))
        qk_pool = ctx.enter_context(tc.tile_pool(name="qk", bufs=HPC))
        qk2_pool = ctx.enter_context(tc.tile_pool(name="qk2", bufs=HPC))
        v_pool = ctx.enter_context(tc.tile_pool(name="v", bufs=NT))
        ctx_pool = ctx.enter_context(tc.tile_pool(name="ctxp", bufs=HPC))
        y_pool = ctx.enter_context(tc.tile_pool(name="y", bufs=3))
        r_pool = ctx.enter_context(tc.tile_pool(name="r", bufs=2))
        r0_pool = ctx.enter_context(tc.tile_pool(name="r0", bufs=2))
        bc_pool = ctx.enter_context(tc.tile_pool(name="bc", bufs=2))
        p_pool = ctx.enter_context(tc.tile_pool(name="p", bufs=6))
        x_pool = ctx.enter_context(tc.tile_pool(name="x", bufs=KT))
        ps_s = ctx.enter_context(tc.tile_pool(name="ps_s", bufs=2, space="PSUM"))
        ps_c = ctx.enter_context(tc.tile_pool(name="ps_c", bufs=2, space="PSUM"))
        ps_a = ctx.enter_context(tc.tile_pool(name="ps_a", bufs=2, space="PSUM"))

        # ---- PE warm-up + exp table preload on a memset tile (no DMA dep).
        # The HAM clock gate needs ~3.4us of sustained matmul activity to lift
        # the PE from 1.2 to 2.4 GHz; spin while the x/weight DMAs land.
        warm_sb = consts.tile([P, 256], bf16)
        nc.vector.memset(warm_sb[:], 0.0)
        wps = ps_a.tile([P, CH], f32, tag="ps_a", name="warm_ps")
        for _ in range(95):
            nc.tensor.matmul(
                wps[:, 0:HD], warm_sb[:, 0:P], warm_sb[:, 0:HD], start=True, stop=True
            )
        actwarm = consts.tile([P, 256], bf16)
        nc.scalar.activation(actwarm[:], warm_sb[:], EXP)

        # ---- tiles for constants (DMAs emitted below in priority order)
        vones_sb = consts.tile([P, VW], bf16)
        wqk_sb = consts.tile([P, KT, 2 * HD * HPC], bf16)
        _wqk = w_qk.rearrange("(t p) m -> p t m", p=P)
        wv_sb = consts.tile([P, KT, VW], bf16)
        bqk_sb = consts.tile([P, HPC], f32)
        wp_sb = consts.tile([HD, HPC, C], bf16)
        onesrow_sb = consts.tile([HD + 1, HD], bf16)
        swapm_sb = consts.tile([P, P], bf16)

        # persistent activations
        qk_sb = [qk_pool.tile([P, N], bf16, tag="qk", name=f"qk{_}") for _ in range(HPC)]
        qk2_sb = [qk2_pool.tile([P, N], bf16, tag="qk2", name=f"qk2_{_}") for _ in range(HPC)]
        v_sb = [v_pool.tile([P, VW], bf16, tag="v", name=f"v{_}") for _ in range(NT)]
        ctx_sb = [ctx_pool.tile([HD, N], bf16, tag="ctx", name=f"ctx{_}") for _ in range(HPC)]
        x_sb = [x_pool.tile([P, N], bf16, tag="x", name=f"x{_}") for _ in range(KT)]

        # x in column-waves: all 6 kt tiles of q-chunk w land together, so the
        # chunk-w qk projection (and v tiles 4w..4w+3) can start before the
        # rest of x arrives.  DMA priority: wave 0 + qkv weights first (they
        # gate the first compute), the late-needed wp/onesrow last.
        def x_wave(w):
            for kt in range(KT):
                nc.sync.dma_start(
                    x_sb[kt][:, w * CH : (w + 1) * CH],
                    xT[kt * P : (kt + 1) * P, w * CH : (w + 1) * CH],
                )

        x_wave(0)
        for kt in range(KT):
            nc.sync.dma_start(wqk_sb[:, kt, :], _wqk[:, kt, :])
        nc.sync.dma_start(wv_sb[:], w_v.rearrange("(t p) m -> p t m", p=P))
        nc.sync.dma_start(vones_sb[:], vones[:])
        nc.sync.dma_start(bqk_sb[:], b_qk.rearrange("t p -> p t"))
        nc.sync.dma_start(swapm_sb[:], swapm[:])

        # ---------- unit emitters ----------
        def emit_qk_group(t, cc, pe_swap=False):
            # qk^T head tile t, q-chunk cc: [q^T(64); k^T(64)] x CH
            sl = slice(cc * CH, (cc + 1) * CH)
            ps = ps_a.tile([P, CH], f32, tag="ps_a", name="ps_qk")
            for kt in range(KT):
                nc.tensor.matmul(
                    ps[:],
                    wqk_sb[:, kt, t * P : (t + 1) * P],
                    x_sb[kt][:, sl],
                    start=(kt == 0),
                    stop=(kt == KT - 1),
                )
            nc.vector.tensor_scalar_add(qk_sb[t][:, sl], ps[:], bqk_sb[:, t : t + 1])
            # swapped copy (k^T to partitions 0:64, q^T to 64:128) so S-pairs
            # can target opposite PE row groups
            if pe_swap:
                sps2 = ps_a.tile([P, CH], f32, tag="ps_a", name="ps_swap")
                nc.tensor.matmul(sps2[:], swapm_sb[:], qk_sb[t][:, sl],
                                 start=True, stop=True)
                nc.vector.tensor_copy(qk2_sb[t][:, sl], sps2[:])
            else:
                nc.sync.dma_start(qk2_sb[t][0:HD, sl], qk_sb[t][HD:P, sl])
                nc.sync.dma_start(qk2_sb[t][HD:P, sl], qk_sb[t][0:HD, sl])

        def emit_v(nt):
            # v natural layout [key, 3*65] (+ ones columns)
            ps = ps_a.tile([P, CH], f32, tag="ps_a", name="ps_v")
            for kt in range(KT):
                nc.tensor.matmul(
                    ps[:, 0:VW],
                    x_sb[kt][:, nt * P : (nt + 1) * P],
                    wv_sb[:, kt, :],
                    start=(kt == 0),
                    stop=(kt == KT - 1),
                )
            nc.vector.tensor_add(v_sb[nt][:], ps[:, 0:VW], vones_sb[:])

        def emit_S_pair(sps, c, h, kp):
            # S^T for key tiles (2kp, 2kp+1) into opposite PE row groups
            kt0, kt1 = 2 * kp, 2 * kp + 1
            nc.tensor.matmul(
                sps[:, 0:CH],
                qk2_sb[h][0:HD, kt0 * P : (kt0 + 1) * P],
                qk_sb[h][0:HD, c * CH : (c + 1) * CH],
            )
            nc.tensor.matmul(
                sps[:, CH : 2 * CH],
                qk_sb[h][HD:P, kt1 * P : (kt1 + 1) * P],
                qk2_sb[h][HD:P, c * CH : (c + 1) * CH],
            )

        def emit_PV(cps, pt, h, kp, first, last):
            kt0, kt1 = 2 * kp, 2 * kp + 1
            nc.tensor.matmul(
                cps[:],
                v_sb[kt0][:, h * 65 : (h + 1) * 65],
                pt[:, 0:CH],
                start=first,
                stop=False,
            )
            nc.tensor.matmul(
                cps[:],
                v_sb[kt1][:, h * 65 : (h + 1) * 65],
                pt[:, CH : 2 * CH],
                start=False,
                stop=last,
            )

        # E projection: per n-tile, psA covers y columns 0:512, psB 512:768.
        ysb_map = {}

        def emit_E_A(nt):
            psA = ps_a.tile([P, CH], f32, tag="ps_a", name="psA")
            for h in range(HPC):
                nc.tensor.matmul(
                    psA[:],
                    ctx_sb[h][:, nt * P : (nt + 1) * P],
                    wp_sb[:, h, 0:CH],
                    start=(h == 0),
                    stop=(h == HPC - 1),
                )
            ysb = y_pool.tile([P, C], bf16, tag="y", name="ysb")
            ysb_map[nt] = ysb
            nc.vector.tensor_copy(ysb[:, 0:CH], psA[:])

        def emit_E_B(nt):
            psB = ps_a.tile([P, CH], f32, tag="ps_a", name="psB")
            for h in range(HPC):
                nc.tensor.matmul(
                    psB[:, 0 : C - CH],
                    ctx_sb[h][:, nt * P : (nt + 1) * P],
                    wp_sb[:, h, CH:C],
                    start=(h == 0),
                    stop=(h == HPC - 1),
                )
            ysb = ysb_map.pop(nt)
            nc.vector.tensor_copy(ysb[:, CH:C], psB[:, 0 : C - CH])
            nc.sync.dma_start(y[nt * P : (nt + 1) * P, :], ysb[:])

        # D normalize, staged: (1) reciprocal of the den row (DVE, PSUM->SBUF),
        # shift it to partition 0 (tiny DMA), broadcast across 64 partitions
        # (GPSIMD); (2) one tensor_mul cps * bc -> ctx (DVE).
        def emit_D1_gpsimd(c, h, cps):
            # den row PSUM->SBUF (f32), reciprocal on SBUF, shift to partition
            # 0 with a tiny DMA, then broadcast on the idle GPSIMD engine.
            dn = r_pool.tile([P, CH], f32, tag="r", name="dn")
            nc.vector.tensor_copy(dn[64:65, :], cps[64:65, :])
            r0 = r0_pool.tile([1, CH], f32, tag="r0", name="r0")
            nc.sync.dma_start(r0[0:1, :], dn[64:65, :])
            r1 = r0_pool.tile([1, CH], f32, tag="r0", name="r1")
            nc.vector.reciprocal_approx_fast(r1[0:1, :], r0[0:1, :])
            bc = bc_pool.tile([HD, CH], f32, tag="bc", name="bc")
            nc.gpsimd.partition_broadcast(bc[:], r1[0:1, :], channels=HD)
            return bc

        # v1-style: den broadcast across partitions via a ones-row matmul,
        # then reciprocal on the broadcast copy.
        def emit_D1_pe(c, h, cps):
            denr = r_pool.tile([P, CH], bf16, tag="r", name="denr")
            nc.vector.tensor_copy(denr[64:65, :], cps[64:65, :])
            bps = ps_a.tile([P, CH], f32, tag="ps_a", name="bps")
            nc.tensor.matmul(
                bps[0:HD, :], onesrow_sb[HD : HD + 1, :], denr[64:65, :],
                start=True, stop=True,
            )
            bcd = bc_pool.tile([HD, CH], f32, tag="bc", name="bcd")
            nc.vector.tensor_copy(bcd[:], bps[0:HD, :])
            bc = bc_pool.tile([HD, CH], f32, tag="bc", name="bc")
            nc.vector.reciprocal_approx_fast(bc[:], bcd[:])
            return bc

        emit_D1 = emit_D1_pe if USE_PE_BCAST else emit_D1_gpsimd

        def emit_D2(c, h, cps, bc):
            nc.vector.tensor_mul(
                ctx_sb[h][:, c * CH : (c + 1) * CH], cps[0:HD, :], bc[:]
            )

        # ---------- schedule ----------
        # c-outer; within a chunk the h0/h1 groups interleave so that during
        # the x-wave-gated first chunk every wave level feeds 4 exps (the
        # engine streams are in-order, so wave-gated work must not sit ahead
        # of ready work).  h2's groups follow, then the next chunk.
        # Slot (h, g) = S-pairs for key tiles 4g..4g+3 of (c, h); the PV
        # consumers lag one slot behind their exp.
        slot_order = [(0, 0), (1, 0), (0, 1), (1, 1), (0, 2), (1, 2),
                      (0, 3), (1, 3), (2, 0), (2, 1), (2, 2), (2, 3)]

        pend_pv = [None]        # (ptA, ptB, h, c, g) awaiting PV in next slot
        pend_D = []             # staged D work: dicts
        pend_E = []             # E halves ready to emit in 64-class slots
        cps_map = {}            # (c, h) -> cps tile being accumulated

        def emit_pv_slot():
            if pend_pv[0] is None:
                return
            ptA, ptB, h, c, g = pend_pv[0]
            pend_pv[0] = None
            if g == 0:
                cps_map[(c, h)] = ps_c.tile([65, CH], f32, tag="ps_c", name="cps")
            cps = cps_map[(c, h)]
            emit_PV(cps, ptA, h, 2 * g, first=(g == 0), last=False)
            emit_PV(cps, ptB, h, 2 * g + 1, first=False, last=(g == NG - 1))
            if g == NG - 1:
                del cps_map[(c, h)]
                pend_D.append({"c": c, "h": h, "cps": cps, "bc": None, "stage": 0})

        def run_D_stage():
            if not pend_D:
                return
            d = pend_D[0]
            if d["stage"] == 0:
                d["bc"] = emit_D1(d["c"], d["h"], d["cps"])
                d["stage"] = 1
            else:
                emit_D2(d["c"], d["h"], d["cps"], d["bc"])
                pend_D.pop(0)
                if d["h"] == HPC - 1:
                    cc = d["c"]
                    for i in range(CH // P):
                        nt = cc * (CH // P) + i
                        pend_E.append(("A", nt))
                        pend_E.append(("B", nt))

        def flush_E(budget):
            while budget > 0 and pend_E:
                kind, nt = pend_E.pop(0)
                if kind == "A":
                    emit_E_A(nt)
                else:
                    emit_E_B(nt)
                budget -= 1

        x_wave(1)
        x_wave(2)
        x_wave(3)
        nc.sync.dma_start(wp_sb[:], w_p.rearrange("(h p) m -> p h m", p=HD))
        nc.sync.dma_start(onesrow_sb[:], onesrow[:])
        # prologue: qk chunk 0 for the interleaved heads (PE-swap: a swap DMA
        # would queue behind the x waves and delay the first S-pairs)
        emit_qk_group(0, 0, pe_swap=True)
        emit_qk_group(1, 0, pe_swap=True)

        for c in range(QCH):
            # per-slot (128,*)-class work for this chunk
            w128 = {si: [] for si in range(len(slot_order))}
            if c == 0:
                w128[0].append(("qkp", 2, 0))        # qk(h2, c0), PE-swap
                for g in range(NG):
                    w128[2 * g].append(("v4", g))    # v tiles 4g..4g+3
            if c < QCH - 1:
                w128[8].append(("qk", 0, c + 1))
                w128[9].append(("qk", 1, c + 1))
                w128[10].append(("qk", 2, c + 1))

            for si, (h, g) in enumerate(slot_order):
                # --- (64,128)-class: 2 S-pairs + E units
                spsA = ps_s.tile([P, 2 * CH], f32, tag="ps_s", name="spsA")
                emit_S_pair(spsA, c, h, 2 * g)
                spsB = ps_s.tile([P, 2 * CH], f32, tag="ps_s", name="spsB")
                emit_S_pair(spsB, c, h, 2 * g + 1)
                flush_E(2)
                # --- ScalarE: exp of both halves
                ptA = p_pool.tile([P, 2 * CH], bf16, tag="p", name="ptA")
                nc.scalar.activation(ptA[:], spsA[:], EXP)
                ptB = p_pool.tile([P, 2 * CH], bf16, tag="p", name="ptB")
                nc.scalar.activation(ptB[:], spsB[:], EXP)
                # --- D chain stage (DVE/PE-bcast only)
                run_D_stage()
                # --- (128,*)-class work
                for kind in w128[si]:
                    if kind[0] == "v4":
                        for nt in range(4 * kind[1], 4 * kind[1] + 4):
                            emit_v(nt)
                    elif kind[0] == "qkp":
                        emit_qk_group(kind[1], kind[2], pe_swap=True)
                    else:
                        emit_qk_group(kind[1], kind[2])
                # --- (128,65)-class: lagged PVs
                emit_pv_slot()
                pend_pv[0] = (ptA, ptB, h, c, g)

        # ---------- drain ----------
        emit_pv_slot()
        while pend_D:
            run_D_stage()
        flush_E(len(pend_E))


def build_program():
    nc = bacc.Bacc("TRN2", target_bir_lowering=False, debug=False)
    xT = nc.dram_tensor("xT", [C, N], bf16, kind="ExternalInput").ap()
    w_qk = nc.dram_tensor("w_qk", [C, 2 * HD * HPC], bf16, kind="ExternalInput").ap()
    w_v = nc.dram_tensor("w_v", [C, VW], bf16, kind="ExternalInput").ap()
    b_qk = nc.dram_tensor("b_qk", [HPC, P], f32, kind="ExternalInput").ap()
    w_p = nc.dram_tensor("w_p", [HPC * HD, C], bf16, kind="ExternalInput").ap()
    vones = nc.dram_tensor("vones", [P, VW], bf16, kind="ExternalInput").ap()
    onesrow = nc.dram_tensor("onesrow", [HD + 1, HD], bf16, kind="ExternalInput").ap()
    swapm = nc.dram_tensor("swapm", [P, P], bf16, kind="ExternalInput").ap()
    y = nc.dram_tensor("y", [N, C], bf16, kind="ExternalOutput").ap()
    with tile.TileContext(nc) as tc:
        _emit(tc, nc, xT, w_qk, w_v, b_qk, w_p, vones, onesrow, swapm, y)
    nc.compile()
    return nc


_CACHE = {}


def _get_program():
    if "nc" not in _CACHE:
        _CACHE["nc"] = build_program()
    return _CACHE["nc"]


def make_in_maps(x, W_qkv, b_qkv, W_proj):
    """Per-core input dicts implementing the (batch, head-group) sharding."""
    x = np.ascontiguousarray(np.asarray(x, np.float32))
    W_qkv = np.asarray(W_qkv, np.float32)
    b_qkv = np.asarray(b_qkv, np.float32)
    W_proj = np.asarray(W_proj, np.float32)
    scale = float(HD) ** -0.5

    Wq = W_qkv[0:C].reshape(H, HD, C)
    Wk = W_qkv[C : 2 * C].reshape(H, HD, C)
    Wv = W_qkv[2 * C : 3 * C].reshape(H, HD, C)
    bq = b_qkv[0:C].reshape(H, HD)
    bk = b_qkv[C : 2 * C].reshape(H, HD)

    vones_mask = np.zeros((P, VW), np.float32)
    for i in range(HPC):
        vones_mask[:, i * 65 + HD] = 1.0
    onesrow_arr = np.zeros((HD + 1, HD), np.float32)
    onesrow_arr[HD, :] = 1.0
    swapm_arr = np.zeros((P, P), np.float32)
    for i in range(P):
        swapm_arr[(i + HD) % P, i] = 1.0

    in_maps = []
    for core in range(NCORES):
        b = core // CORES_PER_B
        hg = core % CORES_PER_B
        heads = list(range(hg * HPC, (hg + 1) * HPC))

        xT = np.ascontiguousarray(x[b].T).astype(ml_dtypes.bfloat16)  # [C, N]
        w_qk = np.empty((C, 2 * HD * HPC), np.float32)  # cast to bf16 below
        b_qk_arr = np.empty((HPC, P), np.float32)
        w_v = np.zeros((C, VW), np.float32)
        w_p = np.empty((HPC * HD, C), np.float32)
        for i, h in enumerate(heads):
            w_qk[:, i * P : i * P + HD] = Wq[h].T * scale
            w_qk[:, i * P + HD : (i + 1) * P] = Wk[h].T
            b_qk_arr[i, 0:HD] = bq[h] * scale
            b_qk_arr[i, HD:P] = bk[h]
            w_v[:, i * 65 : i * 65 + HD] = Wv[h].T
            w_p[i * HD : (i + 1) * HD, :] = W_proj[:, h * HD : (h + 1) * HD].T
        in_maps.append(
            {"xT": xT,
             "w_qk": w_qk.astype(ml_dtypes.bfloat16),
             "w_v": w_v.astype(ml_dtypes.bfloat16),
             "b_qk": b_qk_arr,
             "w_p": w_p.astype(ml_dtypes.bfloat16),
             "vones": vones_mask.astype(ml_dtypes.bfloat16),
             "onesrow": onesrow_arr.astype(ml_dtypes.bfloat16),
             "swapm": swapm_arr.astype(ml_dtypes.bfloat16)}
        )
    return in_maps


def gather_output(results, b_qkv, W_proj, b_proj):
    """Sum the per-core partial projections (TP all-reduce) + effective bias."""
    out = np.zeros((B, N, C), np.float32)
    for core in range(NCORES):
        out[core // CORES_PER_B] += np.asarray(results[core]["y"], np.float32)
    b_v = np.asarray(b_qkv, np.float32)[2 * C : 3 * C]
    b_eff = np.asarray(b_proj, np.float32) + np.asarray(W_proj, np.float32) @ b_v
    out += b_eff
    return out


def kernel(x=None, xpos=None, W_qkv=None, b_qkv=None, W_proj=None, b_proj=None, **kw):
    del xpos, kw  # rope disabled in this configuration; xpos unused
    nc = _get_program()
    in_maps = make_in_maps(x, W_qkv, b_qkv, W_proj)
    res = run_bass_kernel_spmd(nc, in_maps, core_ids=list(range(NCORES)))
    return gather_output(res.results, b_qkv, W_proj, b_proj)
